# revision 25
# baseline (speedup 1.0000x reference)
"""Trainium2 Bass kernel for MeanAggregator GNN message passing.

Computation (see reference):
  h = tanh(BN_trainmode(features @ W.T + b)) ; out = row-mean over sampled
  neighbor set (deduped membership mask) of h rows.  The linear bias b
  cancels exactly inside train-mode BN (shift-invariant), so it is dropped.

Strategy (8 cores, SPMD), rev6 — gather-free, early AllGather stats:
  - Shard OUTPUT rows across cores (512 rows/core).  The host pre-gathers
    the feature rows for each (row, slot) entry: every output row gets
    exactly S=17 slots (pad slots carry weight 0), so each core receives a
    dense [256, 8704] fp16 entry matrix plus a [1, 8704] fp16 weight row.
    This removes the on-device dma_gather (~120us serial GpSimd descriptor
    generation) and the output ReduceScatter (~60us tail) of earlier revs.
  - BN batch stats need the full table: each core computes W @ x^T over a
    6272-row table shard per 512-column chunk and drains per-channel sum
    (DVE reduce) / sumsq (ACT Square accum) straight from PSUM.  The xT
    load is split into interleaved pieces so the GEMM starts ~4us after
    the first piece lands and the collective fires as early as possible.
  - Stats exchange: CC-path AllGather of the [128,2] partials (7 ring
    steps vs 14 for AllReduce; the CC fixed cost is per-step) + local
    slot sum on DVE.  (A direct remote_dma_broadcast SBUF exchange was
    tried and is ms-slow/unreliable under the axon relay.)
  - Entry pipeline: W @ xg^T per 512-entry chunk (PE, fp16, fp32 PSUM);
    the PSUM drain is a single fused ACT pass tanh(ps*scale + shift)
    with per-partition (=per-channel) scale/bias columns; DVE multiplies
    by the broadcast weight row (GpSimd partition_broadcast, no HBM
    cost); 3D-view tensor_reduce sums each row's 17 slots — issued in 4
    row-block checkpoints so the tail reduce is ~1/4 size.
  - Output is [128, 512] (channels x rows) per core; host transposes and
    concatenates.
"""

import os
import sys

# (NEURON_SCRATCHPAD_PAGE_SIZE=4096 was tried: bigger DMA packets but
# the stats collective got ~15us slower — net loss; keep default 256.)

for _p in ("/opt/trn_rl_repo", "/root/.axon_site/_ro/trn_rl_repo"):
    if _p not in sys.path:
        sys.path.append(_p)

import numpy as np

import concourse.bass as bass
import concourse.bacc as bacc
import concourse.tile as tile
import concourse.mybir as mybir
from concourse.bass_utils import run_bass_kernel_spmd

F32 = mybir.dt.float32
F16 = mybir.dt.float16
AF = mybir.ActivationFunctionType
OP = mybir.AluOpType
AX = mybir.AxisListType

N_CORES = 8
U, F, E, B = 50000, 256, 128, 4096
S = 17                  # slots per output row (n_nbr_samples + self)
UL = 6272               # per-core table rows for stats (49 * 128)
R = B // N_CORES        # 512 output rows per core
EN = R * S              # 8704 entries per core (= 17 * 512 exactly)
CH = 512                # entry / table chunk width (one PSUM bank)
BN_EPS = 1e-5

U_CHUNKS = [(i * CH, CH) for i in range(UL // CH)]
if UL % CH:
    U_CHUNKS.append((UL - UL % CH, UL % CH))
E_CHUNKS = [(i * CH, CH) for i in range(EN // CH)]
XT_PIECES = [(0, 1536), (1536, 1536), (3072, 1536), (4608, 1664)]

_CACHE = {}
LAST_RESULTS = None
TRACE = False


def _build():
    if "nc" in _CACHE:
        return _CACHE["nc"]

    nc = bacc.Bacc("TRN2", target_bir_lowering=False, debug=False,
                   enable_asserts=False, num_devices=N_CORES)

    # ---- I/O ----
    xT = nc.dram_tensor("xT", [F, UL], F16, kind="ExternalInput")
    xgT = nc.dram_tensor("xgT", [F, EN], F16, kind="ExternalInput")
    Wt = nc.dram_tensor("Wt", [F, E], F16, kind="ExternalInput")
    gb = nc.dram_tensor("gb", [E, 2], F32, kind="ExternalInput")
    wrow = nc.dram_tensor("wrow", [1, EN], F16, kind="ExternalInput")
    outT = nc.dram_tensor("outT", [E, R], F32, kind="ExternalOutput")

    # ---- internal DRAM (stats exchange via AllToAll: my partial
    # replicated 8x in -> all 8 partials out, single phase) ----
    ag_in = nc.dram_tensor("ag_in", [N_CORES * E, 2], F32)
    ag_out = nc.dram_tensor("ag_out", [N_CORES * E, 2], F32)

    RG = [list(range(N_CORES))]

    with tile.TileContext(nc) as tc:
        with (
            tc.tile_pool(name="const", bufs=1) as cpool,
            tc.tile_pool(name="rot", bufs=3) as rot,
        ):
            # ---- constants / inputs (weight row first: gpsimd broadcast) ----
            wr = cpool.tile([1, EN], F16, tag="wr")
            nc.sync.dma_start(wr[:], wrow[:])
            wt0 = cpool.tile([128, E], F16, tag="wt0")
            wt1 = cpool.tile([128, E], F16, tag="wt1")
            nc.sync.dma_start(wt0[:], Wt[0:128, :])
            nc.sync.dma_start(wt1[:], Wt[128:256, :])
            gbt = cpool.tile([E, 2], F32, tag="gbt")
            nc.sync.dma_start(gbt[:], gb[:])
            epscol = cpool.tile([E, 1], F32, tag="epscol")
            nc.vector.memset(epscol[:], BN_EPS)

            # broadcast weight row to all partitions (GpSimd; idle engine)
            wmt = cpool.tile([128, EN], F16, tag="wmt")
            nc.gpsimd.partition_broadcast(wmt[:], wr[:])

            # table shard in interleaved pieces (stats GEMM starts after
            # piece 0), then entry features
            xt0 = cpool.tile([128, UL], F16, tag="xt0")
            xt1 = cpool.tile([128, UL], F16, tag="xt1")
            for p0, pn in XT_PIECES:
                nc.sync.dma_start(xt0[:, p0:p0 + pn], xT[0:128, p0:p0 + pn])
                nc.sync.dma_start(xt1[:, p0:p0 + pn], xT[128:256, p0:p0 + pn])
            # entry features queue behind the table pieces (same sync
            # queue) so the stats-critical xt load keeps full bandwidth
            xg0 = cpool.tile([128, EN], F16, tag="xg0")
            xg1 = cpool.tile([128, EN], F16, tag="xg1")
            nc.sync.dma_start(xg0[:, 0:EN // 2], xgT[0:128, 0:EN // 2])
            nc.sync.dma_start(xg1[:, 0:EN // 2], xgT[128:256, 0:EN // 2])
            nc.sync.dma_start(xg0[:, EN // 2:], xgT[0:128, EN // 2:])
            nc.sync.dma_start(xg1[:, EN // 2:], xgT[128:256, EN // 2:])

            n_ch = len(U_CHUNKS)
            musum = cpool.tile([E, n_ch], F32, tag="musum")
            ssq = cpool.tile([E, n_ch], F32, tag="ssq")

            # ---- phase A: table GEMM -> per-channel sum / sumsq ----
            with tc.tile_pool(name="psA", bufs=1, space="PSUM") as psA:
                for ci, (u0, un) in enumerate(U_CHUNKS):
                    ps = psA.tile([128, un], F32, tag=f"ps{ci % 2}")
                    nc.tensor.matmul(ps[:], wt0[:], xt0[:, u0:u0 + un],
                                     start=True, stop=False)
                    nc.tensor.matmul(ps[:], wt1[:], xt1[:, u0:u0 + un],
                                     start=False, stop=True)
                    nc.vector.tensor_reduce(musum[:, ci:ci + 1], ps[:],
                                            axis=AX.X, op=OP.add)
                    sqd = rot.tile([128, un], F16, tag="sqd")
                    nc.scalar.activation(sqd[:], ps[:], AF.Square,
                                         accum_out=ssq[:, ci:ci + 1])

            # ---- stats AllGather (7 ring steps; latency hides under
            # the entry DMAs + GEMM) + local slot sum ----
            stats_sb = cpool.tile([E, 2], F32, tag="stats_sb")
            nc.vector.tensor_reduce(stats_sb[:, 0:1], musum[:], axis=AX.X,
                                    op=OP.add)
            nc.vector.tensor_reduce(stats_sb[:, 1:2], ssq[:], axis=AX.X,
                                    op=OP.add)
            for k in range(N_CORES):
                nc.sync.dma_start(ag_in[k * E:(k + 1) * E, :], stats_sb[:])
            nc.gpsimd.collective_compute(
                "AllToAll", OP.bypass, replica_groups=RG,
                ins=[ag_in.ap()], outs=[ag_out.ap()])
            recv = cpool.tile([E, 8, 2], F32, tag="recv")
            nc.sync.dma_start(
                recv[:], ag_out.ap().rearrange("(k p) c -> p k c", p=E))
            stats_g = cpool.tile([E, 2], F32, tag="stats_g")
            nc.vector.tensor_reduce(
                stats_g[:], recv[:].rearrange("p k c -> p c k"),
                axis=AX.X, op=OP.add)

            # ---- per-channel scale/shift (channel == partition: tiny) ----
            mu = cpool.tile([E, 1], F32, tag="mu")
            nc.vector.tensor_scalar_mul(mu[:], stats_g[:, 0:1], 1.0 / U)
            ex2 = cpool.tile([E, 1], F32, tag="ex2")
            nc.vector.tensor_scalar_mul(ex2[:], stats_g[:, 1:2], 1.0 / U)
            musq = cpool.tile([E, 1], F32, tag="musq")
            nc.vector.tensor_tensor(musq[:], mu[:], mu[:], op=OP.mult)
            var = cpool.tile([E, 1], F32, tag="var")
            nc.vector.tensor_tensor(var[:], ex2[:], musq[:], op=OP.subtract)
            sd = cpool.tile([E, 1], F32, tag="sd")
            nc.scalar.activation(sd[:], var[:], AF.Sqrt, bias=epscol[:, 0:1])
            rinv = cpool.tile([E, 1], F32, tag="rinv")
            nc.vector.reciprocal(rinv[:], sd[:])
            scale_c = cpool.tile([E, 1], F32, tag="scale_c")
            nc.vector.tensor_tensor(scale_c[:], rinv[:], gbt[:, 0:1],
                                    op=OP.mult)
            msc = cpool.tile([E, 1], F32, tag="msc")
            nc.vector.tensor_tensor(msc[:], mu[:], scale_c[:], op=OP.mult)
            shift_c = cpool.tile([E, 1], F32, tag="shift_c")
            nc.vector.tensor_tensor(shift_c[:], gbt[:, 1:2], msc[:],
                                    op=OP.subtract)

            # ---- phase B: entry GEMM -> fused BN+tanh drain -> weight ----
            hw = cpool.tile([128, EN], F16, tag="hw")
            outsb = cpool.tile([E, R], F32, tag="outsb")
            # reduce row-block rb (128 rows = 2176 entries) once its
            # entries are drained: after chunks 5, 9, 13, and the end
            ck_after = {(128 * (rb + 1) * S + CH - 1) // CH - 1: rb
                        for rb in range(3)}
            with tc.tile_pool(name="psB", bufs=1, space="PSUM") as psB:
                for ci, (e0, en) in enumerate(E_CHUNKS):
                    ps = psB.tile([128, en], F32, tag=f"pb{ci % 4}")
                    nc.tensor.matmul(ps[:], wt0[:], xg0[:, e0:e0 + en],
                                     start=True, stop=False)
                    nc.tensor.matmul(ps[:], wt1[:], xg1[:, e0:e0 + en],
                                     start=False, stop=True)
                    hn = rot.tile([128, en], F16, tag="hn")
                    nc.scalar.activation(hn[:], ps[:], AF.Tanh,
                                         bias=shift_c[:, 0:1],
                                         scale=scale_c[:, 0:1])
                    nc.vector.tensor_tensor(hw[:, e0:e0 + en], hn[:],
                                            wmt[:, e0:e0 + en], op=OP.mult)
                    rb = ck_after.get(ci)
                    if rb is not None:
                        lo, hi = 128 * rb, 128 * (rb + 1)
                        nc.vector.tensor_reduce(
                            outsb[:, lo:hi],
                            hw[:, lo * S:hi * S].rearrange(
                                "p (r s) -> p r s", s=S),
                            axis=AX.X, op=OP.add)
                nc.vector.tensor_reduce(
                    outsb[:, 384:],
                    hw[:, 384 * S:].rearrange("p (r s) -> p r s", s=S),
                    axis=AX.X, op=OP.add)

            nc.sync.dma_start(outT.ap(), outsb[:])

    nc.compile()
    _CACHE["nc"] = nc
    return nc


def _prep_inputs(features, W, gamma, beta, row_idx, col_idx):
    """Host-side sharding: dedup mask entries, lay out 17 slots per output
    row (zero-weight padding), pre-gather entry feature rows per core."""
    features = np.asarray(features, dtype=np.float32)
    W = np.asarray(W, dtype=np.float32)
    gamma = np.asarray(gamma, dtype=np.float32)
    beta = np.asarray(beta, dtype=np.float32)
    row = np.asarray(row_idx).astype(np.int64)
    col = np.asarray(col_idx).astype(np.int64)

    # dedup (row, col) pairs: mask "set" semantics
    key = row * np.int64(U) + col
    order = np.argsort(key, kind="stable")
    sk = key[order]
    keep_s = np.ones(len(sk), dtype=bool)
    keep_s[1:] = sk[1:] != sk[:-1]
    keep = np.zeros(len(key), dtype=bool)
    keep[order] = keep_s
    urow = row[keep]
    ucol = col[keep]
    cnt = np.bincount(urow, minlength=B)

    # slot layout [B, S]: row r's entries in slots 0..cnt-1, rest weight 0
    o = np.argsort(urow, kind="stable")
    r_s = urow[o]
    c_s = ucol[o]
    cstart = np.concatenate([[0], np.cumsum(cnt)]).astype(np.int64)
    pos = np.arange(len(r_s), dtype=np.int64) - cstart[r_s]
    cols_slot = np.zeros((B, S), dtype=np.int64)
    w_slot = np.zeros((B, S), dtype=np.float32)
    cols_slot[r_s, pos] = c_s
    w_slot[r_s, pos] = 1.0 / np.maximum(cnt, 1)[r_s]

    Wt_full = np.ascontiguousarray(W.T).astype(np.float16)
    gb_full = np.ascontiguousarray(np.stack([gamma, beta], axis=1))

    in_maps = []
    for k in range(N_CORES):
        cf = cols_slot[k * R:(k + 1) * R].reshape(-1)
        wf = w_slot[k * R:(k + 1) * R].reshape(-1).astype(np.float16)
        xgT_k = np.ascontiguousarray(features[cf].T).astype(np.float16)
        lo, hi = k * UL, min((k + 1) * UL, U)
        xpart = np.zeros((UL, F), dtype=np.float32)
        xpart[:hi - lo] = features[lo:hi]
        xT_k = np.ascontiguousarray(xpart.T).astype(np.float16)
        in_maps.append({
            "xT": xT_k,
            "xgT": xgT_k,
            "Wt": Wt_full,
            "gb": gb_full,
            "wrow": wf.reshape(1, EN),
        })
    return in_maps


def kernel(features, W, b, gamma, beta, row_idx, col_idx, B=4096):
    global LAST_RESULTS
    in_maps = _prep_inputs(features, W, gamma, beta, row_idx, col_idx)
    nc = _build()
    res = run_bass_kernel_spmd(nc, in_maps, list(range(N_CORES)), trace=TRACE)
    LAST_RESULTS = res
    out = np.concatenate(
        [np.asarray(res.results[c]["outT"]).T for c in range(N_CORES)],
        axis=0).astype(np.float32)
    return out


# revision 26
# speedup vs baseline: 1.0301x; 1.0301x over previous
"""Trainium2 Bass kernel for MeanAggregator GNN message passing.

Computation (see reference):
  h = tanh(BN_trainmode(features @ W.T + b)) ; out = row-mean over sampled
  neighbor set (deduped membership mask) of h rows.  The linear bias b
  cancels exactly inside train-mode BN (shift-invariant), so it is dropped.

Strategy (8 cores, SPMD), rev8 — gather-free, fp8 stats, early doorbell:
  - Shard OUTPUT rows across cores (512 rows/core).  The host pre-gathers
    the feature rows for each (row, slot) entry: every output row gets
    exactly S=17 slots (pad slots carry weight 0), so each core receives a
    dense [256, 8704] fp16 entry matrix plus a [1, 8704] fp16 weight row.
    This removes the on-device dma_gather (~120us serial GpSimd descriptor
    generation) and the output ReduceScatter (~60us tail) of earlier revs.
  - BN batch stats need the full table; only channel sums/sumsq are used,
    so the table shard + W ride in float8e4 (halves the stats DMA; global
    averaging washes the quantization out — measured 2e-3 end-to-end).
    Per 512-column chunk: matmul to PSUM, DVE reduce -> sum, ACT Square
    accum -> sumsq.  The entry GEMM stays fp16 (its error does NOT
    average: the row-mean shrinks signal as fast as noise).
  - Stats exchange: CC AllGather of [128,2] partials + local slot-sum.
    The CC doorbell quiesces every DMA issued before it in program
    order, so the big entry-feature loads are issued AFTER the
    collective: the doorbell fires at stats-ready (~25us) instead of
    after the xg loads (~45us).  (Direct remote_dma_broadcast SBUF
    exchange was tried: ms-slow + sem-before-data races under axon.)
  - Entry pipeline: W @ xg^T per 512-entry chunk (PE, fp16, fp32 PSUM);
    the PSUM drain is a single fused ACT pass tanh(ps*scale + shift)
    with per-partition (=per-channel) scale/bias columns; DVE multiplies
    by the broadcast weight row (GpSimd partition_broadcast, no HBM
    cost); 3D-view tensor_reduce sums each row's 17 slots in 4
    row-block checkpoints, each followed by its output-piece DMA.
  - Output is [128, 512] (channels x rows) per core; host transposes and
    concatenates.
"""

import sys

for _p in ("/opt/trn_rl_repo", "/root/.axon_site/_ro/trn_rl_repo"):
    if _p not in sys.path:
        sys.path.append(_p)

import ml_dtypes
import numpy as np

import concourse.bass as bass
import concourse.bacc as bacc
import concourse.tile as tile
import concourse.mybir as mybir
from concourse.bass_utils import run_bass_kernel_spmd

F32 = mybir.dt.float32
F16 = mybir.dt.float16
F8 = mybir.dt.float8e4
AF = mybir.ActivationFunctionType
OP = mybir.AluOpType
AX = mybir.AxisListType

N_CORES = 8
U, F, E, B = 50000, 256, 128, 4096
S = 17                  # slots per output row (n_nbr_samples + self)
UL = 6272               # per-core table rows for stats (49 * 128)
R = B // N_CORES        # 512 output rows per core
EN = R * S              # 8704 entries per core (= 17 * 512 exactly)
CH = 512                # entry / table chunk width (one PSUM bank)
BN_EPS = 1e-5

U_CHUNKS = [(i * CH, CH) for i in range(UL // CH)]
if UL % CH:
    U_CHUNKS.append((UL - UL % CH, UL % CH))
E_CHUNKS = [(i * CH, CH) for i in range(EN // CH)]
XT_PIECES = [(0, 1536), (1536, 1536), (3072, 1536), (4608, 1664)]

_CACHE = {}
LAST_RESULTS = None
TRACE = False


def _build():
    if "nc" in _CACHE:
        return _CACHE["nc"]

    nc = bacc.Bacc("TRN2", target_bir_lowering=False, debug=False,
                   enable_asserts=False, num_devices=N_CORES)

    # ---- I/O ----
    xT = nc.dram_tensor("xT", [F, UL], F8, kind="ExternalInput")
    xgT = nc.dram_tensor("xgT", [F, EN], F16, kind="ExternalInput")
    Wt = nc.dram_tensor("Wt", [F, E], F16, kind="ExternalInput")
    W8 = nc.dram_tensor("W8", [F, E], F8, kind="ExternalInput")
    gb = nc.dram_tensor("gb", [E, 2], F32, kind="ExternalInput")
    wrow = nc.dram_tensor("wrow", [1, EN], F16, kind="ExternalInput")
    outT = nc.dram_tensor("outT", [E, R], F32, kind="ExternalOutput")

    # ---- internal DRAM (stats AllGather) ----
    ag_in = nc.dram_tensor("ag_in", [E, 2], F32)
    ag_out = nc.dram_tensor("ag_out", [N_CORES * E, 2], F32,
                            addr_space="Shared")

    RG = [list(range(N_CORES))]

    with tile.TileContext(nc) as tc:
        with (
            tc.tile_pool(name="const", bufs=1) as cpool,
            tc.tile_pool(name="rot", bufs=3) as rot,
        ):
            # ---- constants / inputs (weight row first: gpsimd broadcast) ----
            wr = cpool.tile([1, EN], F16, tag="wr")
            nc.sync.dma_start(wr[:], wrow[:])
            wt80 = cpool.tile([128, E], F8, tag="wt80")
            wt81 = cpool.tile([128, E], F8, tag="wt81")
            nc.sync.dma_start(wt80[:], W8[0:128, :])
            nc.sync.dma_start(wt81[:], W8[128:256, :])
            wt0 = cpool.tile([128, E], F16, tag="wt0")
            wt1 = cpool.tile([128, E], F16, tag="wt1")
            nc.sync.dma_start(wt0[:], Wt[0:128, :])
            nc.sync.dma_start(wt1[:], Wt[128:256, :])
            gbt = cpool.tile([E, 2], F32, tag="gbt")
            nc.sync.dma_start(gbt[:], gb[:])
            epscol = cpool.tile([E, 1], F32, tag="epscol")
            nc.vector.memset(epscol[:], BN_EPS)

            # broadcast weight row to all partitions (GpSimd; idle engine)
            wmt = cpool.tile([128, EN], F16, tag="wmt")
            nc.gpsimd.partition_broadcast(wmt[:], wr[:])

            # table shard (fp8) in interleaved pieces: stats GEMM starts
            # after piece 0
            xt0 = cpool.tile([128, UL], F8, tag="xt0")
            xt1 = cpool.tile([128, UL], F8, tag="xt1")
            for p0, pn in XT_PIECES:
                nc.sync.dma_start(xt0[:, p0:p0 + pn], xT[0:128, p0:p0 + pn])
                nc.sync.dma_start(xt1[:, p0:p0 + pn], xT[128:256, p0:p0 + pn])

            n_ch = len(U_CHUNKS)
            musum = cpool.tile([E, n_ch], F32, tag="musum")
            ssq = cpool.tile([E, n_ch], F32, tag="ssq")

            # ---- phase A: table GEMM (fp8) -> per-channel sum / sumsq ----
            with tc.tile_pool(name="psA", bufs=1, space="PSUM") as psA:
                for ci, (u0, un) in enumerate(U_CHUNKS):
                    ps = psA.tile([128, un], F32, tag=f"ps{ci % 2}")
                    nc.tensor.matmul(ps[:], wt80[:], xt0[:, u0:u0 + un],
                                     start=True, stop=False)
                    nc.tensor.matmul(ps[:], wt81[:], xt1[:, u0:u0 + un],
                                     start=False, stop=True)
                    nc.vector.tensor_reduce(musum[:, ci:ci + 1], ps[:],
                                            axis=AX.X, op=OP.add)
                    sqd = rot.tile([128, un], F16, tag="sqd")
                    nc.scalar.activation(sqd[:], ps[:], AF.Square,
                                         accum_out=ssq[:, ci:ci + 1])

            # ---- stats AllGather; doorbell fires at stats-ready since
            # the entry loads are issued after the collective ----
            stats_sb = cpool.tile([E, 2], F32, tag="stats_sb")
            nc.vector.tensor_reduce(stats_sb[:, 0:1], musum[:], axis=AX.X,
                                    op=OP.add)
            nc.vector.tensor_reduce(stats_sb[:, 1:2], ssq[:], axis=AX.X,
                                    op=OP.add)
            nc.sync.dma_start(ag_in[:], stats_sb[:])
            nc.gpsimd.collective_compute(
                "AllGather", OP.bypass, replica_groups=RG,
                ins=[ag_in.ap()], outs=[ag_out.ap()])

            # entry features (fp16): issued after the collective so the
            # doorbell's DMA quiesce does not cover them; they run during
            # the CC window.  On the sync queue they start once the
            # (stats-gated) ag_in descriptor ahead of them completes.
            xg0 = cpool.tile([128, EN], F16, tag="xg0")
            xg1 = cpool.tile([128, EN], F16, tag="xg1")
            nc.sync.dma_start(xg0[:, 0:EN // 2], xgT[0:128, 0:EN // 2])
            nc.sync.dma_start(xg1[:, 0:EN // 2], xgT[128:256, 0:EN // 2])
            nc.sync.dma_start(xg0[:, EN // 2:], xgT[0:128, EN // 2:])
            nc.sync.dma_start(xg1[:, EN // 2:], xgT[128:256, EN // 2:])

            recv = cpool.tile([E, 8, 2], F32, tag="recv")
            nc.sync.dma_start(
                recv[:], ag_out.ap().rearrange("(k p) c -> p k c", p=E))
            stats_g = cpool.tile([E, 2], F32, tag="stats_g")
            nc.vector.tensor_reduce(
                stats_g[:], recv[:].rearrange("p k c -> p c k"),
                axis=AX.X, op=OP.add)

            # ---- per-channel scale/shift (channel == partition: tiny) ----
            mu = cpool.tile([E, 1], F32, tag="mu")
            nc.vector.tensor_scalar_mul(mu[:], stats_g[:, 0:1], 1.0 / U)
            ex2 = cpool.tile([E, 1], F32, tag="ex2")
            nc.vector.tensor_scalar_mul(ex2[:], stats_g[:, 1:2], 1.0 / U)
            musq = cpool.tile([E, 1], F32, tag="musq")
            nc.vector.tensor_tensor(musq[:], mu[:], mu[:], op=OP.mult)
            var = cpool.tile([E, 1], F32, tag="var")
            nc.vector.tensor_tensor(var[:], ex2[:], musq[:], op=OP.subtract)
            sd = cpool.tile([E, 1], F32, tag="sd")
            nc.scalar.activation(sd[:], var[:], AF.Sqrt, bias=epscol[:, 0:1])
            rinv = cpool.tile([E, 1], F32, tag="rinv")
            nc.vector.reciprocal(rinv[:], sd[:])
            scale_c = cpool.tile([E, 1], F32, tag="scale_c")
            nc.vector.tensor_tensor(scale_c[:], rinv[:], gbt[:, 0:1],
                                    op=OP.mult)
            msc = cpool.tile([E, 1], F32, tag="msc")
            nc.vector.tensor_tensor(msc[:], mu[:], scale_c[:], op=OP.mult)
            shift_c = cpool.tile([E, 1], F32, tag="shift_c")
            nc.vector.tensor_tensor(shift_c[:], gbt[:, 1:2], msc[:],
                                    op=OP.subtract)

            # ---- phase B: entry GEMM -> fused BN+tanh drain -> weight ----
            hw = cpool.tile([128, EN], F16, tag="hw")
            outsb = cpool.tile([E, R], F32, tag="outsb")
            # reduce row-block rb (128 rows = 2176 entries) once its
            # entries are drained, then stream its output piece out
            ck_after = {(128 * (rb + 1) * S + CH - 1) // CH - 1: rb
                        for rb in range(3)}

            def emit_block(rb):
                lo, hi = 128 * rb, 128 * (rb + 1)
                nc.vector.tensor_reduce(
                    outsb[:, lo:hi],
                    hw[:, lo * S:hi * S].rearrange("p (r s) -> p r s", s=S),
                    axis=AX.X, op=OP.add)
                nc.sync.dma_start(outT[:, lo:hi], outsb[:, lo:hi])

            with tc.tile_pool(name="psB", bufs=1, space="PSUM") as psB:
                for ci, (e0, en) in enumerate(E_CHUNKS):
                    ps = psB.tile([128, en], F32, tag=f"pb{ci % 4}")
                    nc.tensor.matmul(ps[:], wt0[:], xg0[:, e0:e0 + en],
                                     start=True, stop=False)
                    nc.tensor.matmul(ps[:], wt1[:], xg1[:, e0:e0 + en],
                                     start=False, stop=True)
                    hn = rot.tile([128, en], F16, tag="hn")
                    nc.scalar.activation(hn[:], ps[:], AF.Tanh,
                                         bias=shift_c[:, 0:1],
                                         scale=scale_c[:, 0:1])
                    nc.vector.tensor_tensor(hw[:, e0:e0 + en], hn[:],
                                            wmt[:, e0:e0 + en], op=OP.mult)
                    rb = ck_after.get(ci)
                    if rb is not None:
                        emit_block(rb)
                emit_block(3)

    nc.compile()
    _CACHE["nc"] = nc
    return nc


def _prep_inputs(features, W, gamma, beta, row_idx, col_idx):
    """Host-side sharding: dedup mask entries, lay out 17 slots per output
    row (zero-weight padding), pre-gather entry feature rows per core."""
    features = np.asarray(features, dtype=np.float32)
    W = np.asarray(W, dtype=np.float32)
    gamma = np.asarray(gamma, dtype=np.float32)
    beta = np.asarray(beta, dtype=np.float32)
    row = np.asarray(row_idx).astype(np.int64)
    col = np.asarray(col_idx).astype(np.int64)

    # dedup (row, col) pairs: mask "set" semantics
    key = row * np.int64(U) + col
    order = np.argsort(key, kind="stable")
    sk = key[order]
    keep_s = np.ones(len(sk), dtype=bool)
    keep_s[1:] = sk[1:] != sk[:-1]
    keep = np.zeros(len(key), dtype=bool)
    keep[order] = keep_s
    urow = row[keep]
    ucol = col[keep]
    cnt = np.bincount(urow, minlength=B)

    # slot layout [B, S]: row r's entries in slots 0..cnt-1, rest weight 0
    o = np.argsort(urow, kind="stable")
    r_s = urow[o]
    c_s = ucol[o]
    cstart = np.concatenate([[0], np.cumsum(cnt)]).astype(np.int64)
    pos = np.arange(len(r_s), dtype=np.int64) - cstart[r_s]
    cols_slot = np.zeros((B, S), dtype=np.int64)
    w_slot = np.zeros((B, S), dtype=np.float32)
    cols_slot[r_s, pos] = c_s
    w_slot[r_s, pos] = 1.0 / np.maximum(cnt, 1)[r_s]

    Wt_full = np.ascontiguousarray(W.T).astype(np.float16)
    W8_full = np.ascontiguousarray(W.T).astype(ml_dtypes.float8_e4m3)
    gb_full = np.ascontiguousarray(np.stack([gamma, beta], axis=1))

    in_maps = []
    for k in range(N_CORES):
        cf = cols_slot[k * R:(k + 1) * R].reshape(-1)
        wf = w_slot[k * R:(k + 1) * R].reshape(-1).astype(np.float16)
        xgT_k = np.ascontiguousarray(features[cf].T).astype(np.float16)
        lo, hi = k * UL, min((k + 1) * UL, U)
        xpart = np.zeros((UL, F), dtype=np.float32)
        xpart[:hi - lo] = features[lo:hi]
        xT_k = np.ascontiguousarray(xpart.T).astype(ml_dtypes.float8_e4m3)
        in_maps.append({
            "xT": xT_k,
            "xgT": xgT_k,
            "Wt": Wt_full,
            "W8": W8_full,
            "gb": gb_full,
            "wrow": wf.reshape(1, EN),
        })
    return in_maps


def kernel(features, W, b, gamma, beta, row_idx, col_idx, B=4096):
    global LAST_RESULTS
    in_maps = _prep_inputs(features, W, gamma, beta, row_idx, col_idx)
    nc = _build()
    res = run_bass_kernel_spmd(nc, in_maps, list(range(N_CORES)), trace=TRACE)
    LAST_RESULTS = res
    out = np.concatenate(
        [np.asarray(res.results[c]["outT"]).T for c in range(N_CORES)],
        axis=0).astype(np.float32)
    return out
